# revision 10
# baseline (speedup 1.0000x reference)
"""FP8 GEMM kernel (MixLinear) for 8 trn2 NeuronCores.

Reference computation:
    s      = max(|x|) / 448                        (global fp32 scalar)
    q_x    = e4m3fn(clip(x / s, +-448))            (OCP e4m3fn)
    q_w    = e4m3fn(clip(w, +-448))                (scale_weight = 1)
    y      = (q_x @ q_w.T) * s + bias              (fp32 accum -> fp16)

Strategy: data-parallel over the 16384 token rows (2048 rows per core).
Host does layout only; device does amax, a cross-core max-exchange,
quantization, DoubleRow fp8 matmul and scale+bias eviction.

TRN e4m3 tops out at 240 (vs OCP 448), so x is quantized at half scale:
    q_half = trn_e4m3(x * (224/gmax))  ==  ocp_e4m3(x / s) / 2
exactly for all magnitudes >= 2^-6 * s; weights (|w| <= 1/sqrt(2048))
are in the range where the TRN and OCP grids agree exactly, so they are
quantized at scale 1.  The output scale is then 2*s = gmax/224.

Key schedule decisions (from v1-v3 traces):
  - The CC-engine AllGather path for the global max has a hard floor:
    its start is max(trigger, NEFF-init-barrier-end ~60-72us) + ~11.5us
    arming + ~17-20us latency => scale ready no earlier than ~90us.
    Instead the 8 per-core max vectors are combined with a 3-step
    XOR-hypercube exchange over remote_dma_broadcast (hardware SDMA +
    semaphores, no ncfw involvement): partner = me XOR {1,2,4},
    relative destinations, ~1-2us per hop.  Launch skew between cores
    is well under the ~25us margin before the first send, so the
    peers' preamble semaphore-clears are long done by the time data
    arrives (the usual reason for a pre-exchange barrier).
  - DMA transfers drain a queue in instruction order, so the x tiles
    are nosync-dep-chained and the weight loads chained behind them on
    the same sync queue: x owns the HBM port until the amax input is
    complete, weights immediately after.
  - amax: DVE tensor_reduce sustains only ~115 G elem/s, so GPSIMD
    (XYZWC cross-lane reduce, ~72 G elem/s) takes every third half-tile.
  - Matmul phase runs j-outer over mt-pairs for mt0-3 (consumes weight
    casts as the ACT engine produces them), then nt-outer/j-inner with
    per-bank eviction pipelining; quantization is emitted interleaved
    in exact consumption order so DVE priority matches need.
"""

import numpy as np

B, S, D_IN, D_OUT = 2, 8192, 2048, 2048
N_CORES = 8
TOK = B * S                  # 16384
TOK_PC = TOK // N_CORES      # 2048 token rows per core
P = 128
KP = D_IN // (2 * P)         # 8 k-pairs of 256 (DoubleRow granularity)
MT = TOK_PC // P             # 16 token tiles per core
N_TILE = 512
NT = D_OUT // N_TILE         # 4 output column tiles
HTOK = TOK_PC // 2           # 1024-token half tiles for amax pipelining

_compiled = None


def _build():
    import concourse.bacc as bacc
    import concourse.tile as tile
    from concourse import mybir
    from concourse.bass import _add_dep_helper
    from concourse.masks import make_identity

    f16 = mybir.dt.float16
    f32 = mybir.dt.float32
    f8 = mybir.dt.float8e4
    Alu = mybir.AluOpType
    Axis = mybir.AxisListType
    Act = mybir.ActivationFunctionType

    nc = bacc.Bacc("TRN2", target_bir_lowering=False, debug=False,
                   num_devices=N_CORES)

    # xt: x^T shard [d_in, tok_pc]; wt: w^T [d_in, d_out] (replicated)
    xt = nc.dram_tensor("xt", [D_IN, TOK_PC], f16, kind="ExternalInput")
    wt = nc.dram_tensor("wt", [D_IN, D_OUT], f16, kind="ExternalInput")
    bias = nc.dram_tensor("bias", [D_OUT], f16, kind="ExternalInput")
    y = nc.dram_tensor("y", [TOK_PC, D_OUT], f16, kind="ExternalOutput")

    def chain(inst, prev, why):
        if prev is not None:
            _add_dep_helper(inst.ins, prev.ins, sync=True, reason=why)
        return inst

    with tile.TileContext(nc) as tc:
        with (
            tc.tile_pool(name="xpool", bufs=KP) as xpool,
            tc.tile_pool(name="qxpool", bufs=KP) as qxpool,
            tc.tile_pool(name="qwpool", bufs=KP) as qwpool,
            tc.tile_pool(name="wstage", bufs=3) as wstage,
            tc.tile_pool(name="small", bufs=1) as small,
            tc.tile_pool(name="ypool", bufs=8) as ypool,
            tc.tile_pool(name="psum", bufs=8, space="PSUM") as psum,
        ):
            # ---- Phase A: x load (chained => exclusive HBM port, fixed
            # arrival order), abs-max per half-tile as each half lands.
            # Arrival i goes to GPSIMD when i%3==2, else DVE.
            pmax = small.tile([P, 2 * KP], f16)
            nc.vector.memset(pmax[:], 0.0)
            grow = small.tile([1, 2 * KP], f32)
            nc.gpsimd.memset(grow[:], 0.0)
            x_sb = []
            prev_dma = None
            for j in range(KP):
                t = xpool.tile([P, 2, TOK_PC], f16, tag="xsb")
                src = xt[2 * j * P:(2 * j + 2) * P, :]
                src = src.rearrange("(p t) m -> p t m", t=2)
                for h in range(2):
                    i = 2 * j + h
                    sl = slice(0, HTOK) if h == 0 else slice(HTOK, TOK_PC)
                    dma = nc.sync.dma_start(t[:, :, sl], src[:, :, sl])
                    prev_dma = chain(dma, prev_dma, "x-order")
                    if i % 3 == 2:
                        nc.gpsimd.tensor_reduce(
                            out=grow[:, i:i + 1], in_=t[:, :, sl],
                            axis=Axis.XYZWC, op=Alu.max,
                            apply_absolute_value=True)
                    else:
                        nc.vector.tensor_reduce(
                            out=pmax[:, i:i + 1], in_=t[:, :, sl],
                            axis=Axis.XY, op=Alu.max,
                            apply_absolute_value=True)
                x_sb.append(t)

            # ---- weights: chained behind x on the same queue; casts to
            # fp8 on ACT (j0-5) and DVE (j6-7, emitted before quant so
            # they beat the matmul's j6/j7 consumption) ----
            qw = []
            wstages = []
            for j in range(KP):
                stage = wstage.tile([P, 2, D_OUT], f16, tag="wst")
                src = wt[2 * j * P:(2 * j + 2) * P, :]
                dma = nc.sync.dma_start(stage[:],
                                        src.rearrange("(p t) n -> p t n", t=2))
                prev_dma = chain(dma, prev_dma, "w-after-x")
                qt = qwpool.tile([P, 2, D_OUT], f8, tag="qw")
                if j < 6:
                    nc.scalar.activation(qt[:], stage[:], Act.Copy)
                else:
                    nc.vector.tensor_scalar(out=qt[:], in0=stage[:],
                                            scalar1=1.0, scalar2=None,
                                            op0=Alu.mult)
                qw.append(qt)
                wstages.append(stage)

            # bias + identity (small, off the critical path)
            ident = small.tile([P, P], f32)
            make_identity(nc, ident[:])
            bias_row = small.tile([1, D_OUT], f16)
            nc.scalar.dma_start(bias_row[:], bias[None, :])
            bias_bc = small.tile([P, D_OUT], f16)
            nc.gpsimd.partition_broadcast(bias_bc[:], bias_row[:], P)

            # ---- fold local partials into one [128,1] vector ----
            cur = small.tile([P, 1], f32, name="cur0")
            nc.vector.tensor_reduce(out=cur[:], in_=pmax[:], axis=Axis.X,
                                    op=Alu.max)
            g0 = small.tile([1, 1], f32)
            nc.gpsimd.tensor_reduce(out=g0[:], in_=grow[:], axis=Axis.XYZWC,
                                    op=Alu.max)
            nc.vector.tensor_tensor(out=cur[0:1, 0:1], in0=cur[0:1, 0:1],
                                    in1=g0[:], op=Alu.max)

            # ---- Phase B: 3-step XOR-hypercube max exchange over
            # remote_dma (hardware SDMA + semaphores; no CC engine, no
            # ncfw, no init-barrier dependency).  Step k: send cur to
            # core (me XOR 2^k) at dest slot 2^k (slot bit2 selects the
            # D2D-capable engines for the cross-die hop), wait for the
            # partner's 2 sem increments, combine.  The body lives in a
            # tile_critical section: the scheduler places it as one
            # unit (pre/post deps on all body tensors), and the raw
            # in-order Pool stream makes wait_ge -> combine safe.
            inboxes = [small.tile([P, 1], f32, name=f"inbox{k}")
                       for k in range(3)]
            curs = [small.tile([P, 1], f32, name=f"cur{k + 1}")
                    for k in range(3)]
            rsems = [nc.alloc_semaphore(f"xmax_rsem{k}") for k in range(3)]
            lsem = nc.alloc_semaphore("xmax_lsem")
            psem = nc.alloc_semaphore("xmax_psem")
            vsem = nc.alloc_semaphore("xmax_vsem")
            with tc.tile_critical(name="xmax"):
                # Pool: descgen + trigger; Vector: wait + max-combine.
                # Cross-engine handoffs via explicit sems (raw streams).
                for k, delta in enumerate((1, 2, 4)):
                    rd: list = [None] * 8
                    rd[delta] = (0, delta)
                    if k > 0:
                        nc.gpsimd.wait_ge(vsem, k)
                    nc.gpsimd.remote_dma_broadcast(
                        inboxes[k][:], cur[:], rsems[k], lsem,
                        rdests=rd).then_inc(psem, 1)
                    nc.gpsimd.wait_ge(psem, k + 1)
                    nc.gpsimd.trigger_dma(count=1)
                    nc.vector.wait_ge(rsems[k], 2)
                    nc.vector.tensor_tensor(
                        out=curs[k][:], in0=cur[:], in1=inboxes[k][:],
                        op=Alu.max).then_inc(vsem, 1)
                    cur = curs[k]
            curf = small.tile([P, 1], f32, name="curf")
            nc.vector.tensor_copy(curf[:], curs[2][:])

            # ---- scalar global max + scales ----
            lmax_t = psum.tile([1, P], f32, tag="ps", name="lmaxt")
            nc.tensor.transpose(lmax_t[:], curf[:], ident[:])
            gmax0 = small.tile([1, 1], f32)
            nc.vector.tensor_reduce(out=gmax0[:], in_=lmax_t[:], axis=Axis.X,
                                    op=Alu.max)
            # scale math on partition 0: col0 = inv_half, col1 = out_scale
            sc = small.tile([1, 2], f32)
            nc.vector.reciprocal(sc[:, 0:1], gmax0[:])
            nc.vector.tensor_scalar_mul(sc[:, 0:1], sc[:, 0:1], 224.0)
            nc.vector.tensor_scalar_mul(sc[:, 1:2], gmax0[:], 1.0 / 224.0)
            scales = small.tile([P, 2], f32)
            nc.gpsimd.partition_broadcast(scales[:], sc[:], P)
            inv_half = scales[:, 0:1]
            out_scale = scales[:, 1:2]

            # ---- Phases C+D interleaved: quantize (DVE) in exact
            # consumption order; matmul j-outer over the first two
            # mt-pairs (weight casts land j-progressively), nt-outer
            # afterwards; evict each PSUM bank as it completes ----
            qx = [qxpool.tile([P, 2, TOK_PC], f8, tag="qx", name=f"qx{j}")
                  for j in range(KP)]

            def quant(j, mt0, nmt):
                sl = slice(mt0 * P, (mt0 + nmt) * P)
                nc.vector.tensor_scalar(out=qx[j][:, :, sl],
                                        in0=x_sb[j][:, :, sl],
                                        scalar1=inv_half[:, 0:1],
                                        scalar2=None, op0=Alu.mult)

            def evict(mt, nt, ps):
                ysb = ypool.tile([P, N_TILE], f16, tag="ysb", name="ysb")
                nc.vector.scalar_tensor_tensor(
                    out=ysb[:], in0=ps[:], scalar=out_scale[:, 0:1],
                    in1=bias_bc[:, nt * N_TILE:(nt + 1) * N_TILE],
                    op0=Alu.mult, op1=Alu.add)
                nc.sync.dma_start(
                    y[mt * P:(mt + 1) * P, nt * N_TILE:(nt + 1) * N_TILE],
                    ysb[:])

            def mm(ps, mt, nt, j):
                nc.tensor.matmul(
                    ps[:],
                    qx[j][:, :, mt * P:(mt + 1) * P],
                    qw[j][:, :, nt * N_TILE:(nt + 1) * N_TILE],
                    start=(j == 0), stop=(j == KP - 1),
                    perf_mode=mybir.MatmulPerfMode.DoubleRow)

            # quant for mt0..3, j-major (matches pair j-outer consumption)
            for mts in ((0, 1), (2, 3)):
                for j in range(KP):
                    for mt in mts:
                        quant(j, mt, 1)

            for pi, mts in enumerate(((0, 1), (2, 3))):
                pss = {(mt, nt): psum.tile([P, N_TILE], f32, tag="ps",
                                           name=f"p{mt}_{nt}")
                       for mt in mts for nt in range(NT)}
                for j in range(KP):
                    for mt in mts:
                        for nt in range(NT):
                            mm(pss[(mt, nt)], mt, nt, j)
                for mt in mts:
                    for nt in range(NT):
                        evict(mt, nt, pss[(mt, nt)])
                # look-ahead quant for the next pair of mt-pairs
                for mt in (4, 5) if pi == 0 else (6, 7):
                    for j in range(KP):
                        quant(j, mt, 1)

            for mt in range(4, MT):
                for nt in range(NT):
                    ps = psum.tile([P, N_TILE], f32, tag="ps", name=f"ps{nt}")
                    for j in range(KP):
                        mm(ps, mt, nt, j)
                    evict(mt, nt, ps)
                if mt + 4 < MT:
                    for j in range(KP):
                        quant(j, mt + 4, 1)

    nc.compile()
    return nc


def _get_compiled():
    global _compiled
    if _compiled is None:
        _compiled = _build()
    return _compiled


def run(x, weight, bias, **kw):
    """Shard + run on 8 cores; returns (full_output, BassKernelResults)."""
    from concourse.bass_utils import run_bass_kernel_spmd

    nc = _get_compiled()

    x = np.asarray(x, dtype=np.float16)
    weight = np.asarray(weight, dtype=np.float16)
    bias = np.asarray(bias, dtype=np.float16)
    xt = np.ascontiguousarray(x.reshape(TOK, D_IN).T)          # [d_in, tok]
    wt = np.ascontiguousarray(weight.T)                        # [d_in, d_out]
    in_maps = []
    for i in range(N_CORES):
        in_maps.append({
            "xt": np.ascontiguousarray(xt[:, i * TOK_PC:(i + 1) * TOK_PC]),
            "wt": wt,
            "bias": bias,
        })
    res = run_bass_kernel_spmd(nc, in_maps, core_ids=list(range(N_CORES)), **kw)
    out = np.concatenate([res.results[i]["y"] for i in range(N_CORES)], axis=0)
    return out.reshape(B, S, D_OUT), res


def kernel(x, weight, bias):
    out, _ = run(x, weight, bias)
    return out


# revision 11
# speedup vs baseline: 26.3554x; 26.3554x over previous
"""FP8 GEMM kernel (MixLinear) for 8 trn2 NeuronCores.

Reference computation:
    s      = max(|x|) / 448                        (global fp32 scalar)
    q_x    = e4m3fn(clip(x / s, +-448))            (OCP e4m3fn)
    q_w    = e4m3fn(clip(w, +-448))                (scale_weight = 1)
    y      = (q_x @ q_w.T) * s + bias              (fp32 accum -> fp16)

Strategy: data-parallel over the 16384 token rows (2048 rows per core).
Host does layout only; device does amax, a cross-core max-exchange,
quantization, DoubleRow fp8 matmul and scale+bias eviction.

TRN e4m3 tops out at 240 (vs OCP 448), so x is quantized at half scale:
    q_half = trn_e4m3(x * (224/gmax))  ==  ocp_e4m3(x / s) / 2
exactly for all magnitudes >= 2^-6 * s; weights (|w| <= 1/sqrt(2048))
are in the range where the TRN and OCP grids agree exactly, so they are
quantized at scale 1.  The output scale is then 2*s = gmax/224.

Key schedule decisions (from the v1-v4 traces):
  - The CC-engine AllGather path for the global max has a hard floor
    (ncfw init barrier + ~11.5us arming + ~17us latency => scale ready
    ~90us+).  Instead every core broadcasts its per-partition |x|-max
    vector to all 8 cores with ONE remote_dma_broadcast (hardware SDMA
    + semaphores, rdests=[(0,k) for k in 8], destination column
    selected by partition_id), waits for 16 remote-sem increments (2
    per sender) and max-reduces the gathered [128,8] block.
  - A trailing dummy CC AllGather (triggered off the critical path,
    completing under the matmul phase) keeps a collective in the NEFF:
    without one the runtime launches the 8 cores unsynchronized
    (~multi-ms skew observed), with one the starts line up to ~10us.
  - DMA rings execute descriptors in instruction order, so x tiles and
    then weights are chained with NOSYNC deps on one queue: ordering
    without the serialization penalty of a semaphore chain (sync=True
    chaining measured 124 GB/s; pipelined ring sustains ~358 GB/s).
  - amax: DVE tensor_reduce sustains ~115 G elem/s, GPSIMD cross-lane
    ~70 G elem/s; half-tiles are split between them by arrival index.
  - Weight fp8 casts: ACT takes j0-5 (emitted early), DVE takes j6-7
    but emitted after the exchange so DVE never stalls the scale path.
  - Matmul runs j-outer over mt-pairs for mt0-3 (consumes weight casts
    as they land), then nt-outer/j-inner with per-bank eviction
    pipelining; quantization is emitted in exact consumption order.
"""

import numpy as np

B, S, D_IN, D_OUT = 2, 8192, 2048, 2048
N_CORES = 8
TOK = B * S                  # 16384
TOK_PC = TOK // N_CORES      # 2048 token rows per core
P = 128
KP = D_IN // (2 * P)         # 8 k-pairs of 256 (DoubleRow granularity)
MT = TOK_PC // P             # 16 token tiles per core
N_TILE = 512
NT = D_OUT // N_TILE         # 4 output column tiles
HTOK = TOK_PC // 2           # 1024-token half tiles for amax pipelining
GPS_CHUNKS = {1, 3, 5, 7, 9, 11}   # arrival indices reduced on GPSIMD

_compiled = None


def _build():
    import concourse.bacc as bacc
    import concourse.tile as tile
    from concourse import mybir
    from concourse.bass import DynSlice, _add_dep_helper
    from concourse.masks import make_identity

    f16 = mybir.dt.float16
    f32 = mybir.dt.float32
    f8 = mybir.dt.float8e4
    Alu = mybir.AluOpType
    Axis = mybir.AxisListType
    Act = mybir.ActivationFunctionType

    nc = bacc.Bacc("TRN2", target_bir_lowering=False, debug=False,
                   num_devices=N_CORES)

    # xt: x^T shard [d_in, tok_pc]; wt: w^T [d_in, d_out] (replicated)
    xt = nc.dram_tensor("xt", [D_IN, TOK_PC], f16, kind="ExternalInput")
    wt = nc.dram_tensor("wt", [D_IN, D_OUT], f16, kind="ExternalInput")
    bias = nc.dram_tensor("bias", [D_OUT], f16, kind="ExternalInput")
    y = nc.dram_tensor("y", [TOK_PC, D_OUT], f16, kind="ExternalOutput")

    # bounce buffers for the trailing launch-sync AllGather
    cc_in = nc.dram_tensor("cc_in", [16], f32)
    cc_out = nc.dram_tensor("cc_out", [16 * N_CORES], f32, addr_space="Shared")
    groups = [list(range(N_CORES))]

    def order(inst, prev, why):
        if prev is not None:
            _add_dep_helper(inst.ins, prev.ins, sync=False, reason=why)
        return inst

    with tile.TileContext(nc) as tc:
        with (
            tc.tile_pool(name="xpool", bufs=KP) as xpool,
            tc.tile_pool(name="qxpool", bufs=KP) as qxpool,
            tc.tile_pool(name="qwpool", bufs=KP) as qwpool,
            tc.tile_pool(name="wstage", bufs=3) as wstage,
            tc.tile_pool(name="small", bufs=1) as small,
            tc.tile_pool(name="ypool", bufs=8) as ypool,
            tc.tile_pool(name="psum", bufs=8, space="PSUM") as psum,
        ):
            # ---- Phase A: x load (nosync-ordered on one ring), amax
            # per half-tile as each half lands, split DVE/GPSIMD ----
            pmax = small.tile([P, 2 * KP], f16)
            nc.vector.memset(pmax[:], 0.0)
            grow = small.tile([1, 2 * KP], f32)
            nc.gpsimd.memset(grow[:], 0.0)
            x_sb = []
            prev_dma = None
            for j in range(KP):
                t = xpool.tile([P, 2, TOK_PC], f16, tag="xsb")
                src = xt[2 * j * P:(2 * j + 2) * P, :]
                src = src.rearrange("(p t) m -> p t m", t=2)
                for h in range(2):
                    i = 2 * j + h
                    sl = slice(0, HTOK) if h == 0 else slice(HTOK, TOK_PC)
                    dma = nc.sync.dma_start(t[:, :, sl], src[:, :, sl])
                    prev_dma = order(dma, prev_dma, "x-order")
                    if i in GPS_CHUNKS:
                        nc.gpsimd.tensor_reduce(
                            out=grow[:, i:i + 1], in_=t[:, :, sl],
                            axis=Axis.XYZWC, op=Alu.max,
                            apply_absolute_value=True)
                    else:
                        nc.vector.tensor_reduce(
                            out=pmax[:, i:i + 1], in_=t[:, :, sl],
                            axis=Axis.XY, op=Alu.max,
                            apply_absolute_value=True)
                x_sb.append(t)

            # ---- weights: ring-ordered behind x; ACT casts j0-5 here,
            # DVE casts j6-7 after the exchange (so DVE stays clear) ----
            qw = []
            wstages = []
            for j in range(KP):
                stage = wstage.tile([P, 2, D_OUT], f16, tag="wst")
                src = wt[2 * j * P:(2 * j + 2) * P, :]
                dma = nc.sync.dma_start(stage[:],
                                        src.rearrange("(p t) n -> p t n", t=2))
                prev_dma = order(dma, prev_dma, "w-after-x")
                qt = qwpool.tile([P, 2, D_OUT], f8, tag="qw")
                if j < 6:
                    nc.scalar.activation(qt[:], stage[:], Act.Copy)
                qw.append(qt)
                wstages.append(stage)

            # bias broadcast early on gpsimd (shares the resident ucode
            # lib with the cross-lane reduces); identity + ones for the
            # PE-side folds
            ident = small.tile([P, P], f32)
            make_identity(nc, ident[:])
            bias_row = small.tile([1, D_OUT], f16)
            nc.scalar.dma_start(bias_row[:], bias[None, :])
            bias_bc = small.tile([P, D_OUT], f16)
            nc.gpsimd.partition_broadcast(bias_bc[:], bias_row[:], P)
            ones_row = small.tile([1, P], f32)
            nc.vector.memset(ones_row[:], 1.0)

            # ---- fold local partials into one [128,1] vector ----
            cur0 = small.tile([P, 1], f32, name="cur0")
            nc.vector.tensor_reduce(out=cur0[:], in_=pmax[:], axis=Axis.X,
                                    op=Alu.max)
            g0 = small.tile([1, 1], f32)
            nc.gpsimd.tensor_reduce(out=g0[:], in_=grow[:], axis=Axis.XYZWC,
                                    op=Alu.max)
            nc.vector.tensor_tensor(out=cur0[0:1, 0:1], in0=cur0[0:1, 0:1],
                                    in1=g0[:], op=Alu.max)

            # ---- Phase B: one-shot all-to-all max gather over
            # remote_dma_broadcast (hardware SDMA + semaphores): every
            # core sends cur0 to column `me` of every core's inbox,
            # waits for all 8 senders (2 sem incs each), reduces. ----
            inbox = small.tile([P, N_CORES], f32, name="inbox")
            gv = small.tile([P, 1], f32, name="gv")
            rsem = nc.alloc_semaphore("xmax_rsem")
            lsem = nc.alloc_semaphore("xmax_lsem")
            psem = nc.alloc_semaphore("xmax_psem")
            with tc.tile_critical(name="xmax"):
                me = nc.gpsimd.partition_id()
                rd = [(0, k) for k in range(N_CORES)]
                nc.gpsimd.remote_dma_broadcast(
                    inbox[:, DynSlice(me, 1)], cur0[:], rsem, lsem,
                    rdests=rd).then_inc(psem, 1)
                nc.gpsimd.wait_ge(psem, 1)
                nc.gpsimd.trigger_dma(count=1)
                nc.vector.wait_ge(rsem, 2 * N_CORES)
                nc.vector.tensor_reduce(out=gv[:], in_=inbox[:], axis=Axis.X,
                                        op=Alu.max)

            # ---- scalar global max + scales (PE transpose fold, PE
            # ones-outer-product broadcast; gpsimd keeps its SWDGE lib)
            lmax_t = psum.tile([1, P], f32, tag="ps", name="lmaxt")
            nc.tensor.transpose(lmax_t[:], gv[:], ident[:])
            gmax0 = small.tile([1, 1], f32)
            nc.vector.tensor_reduce(out=gmax0[:], in_=lmax_t[:], axis=Axis.X,
                                    op=Alu.max)
            # scale math on partition 0: col0 = inv_half, col1 = out_scale
            sc = small.tile([1, 2], f32)
            nc.vector.reciprocal(sc[:, 0:1], gmax0[:])
            nc.vector.tensor_scalar_mul(sc[:, 0:1], sc[:, 0:1], 224.0)
            nc.vector.tensor_scalar_mul(sc[:, 1:2], gmax0[:], 1.0 / 224.0)
            scps = psum.tile([P, 2], f32, tag="ps", name="scps")
            nc.tensor.matmul(scps[:], ones_row[:], sc[:], start=True,
                             stop=True)
            scales = small.tile([P, 2], f32)
            nc.vector.tensor_copy(scales[:], scps[:])
            inv_half = scales[:, 0:1]
            out_scale = scales[:, 1:2]

            # DVE weight casts for j6/j7 (ready ~56-60us, consumed at
            # ~64-66us by the pair loop)
            for j in (6, 7):
                nc.vector.tensor_scalar(out=qw[j][:], in0=wstages[j][:],
                                        scalar1=1.0, scalar2=None,
                                        op0=Alu.mult)

            # trailing dummy collective for synchronized launch; the
            # cc_in DMA depends on scales so it cannot float early and
            # block gpsimd; completes well inside the matmul phase.
            nc.scalar.dma_start(cc_in[0:2], scales[0:1, :])
            nc.gpsimd.collective_compute(
                "AllGather", Alu.bypass, replica_groups=groups,
                ins=[cc_in.ap().opt()], outs=[cc_out.ap().opt()])

            # ---- Phases C+D interleaved: quantize (DVE) in exact
            # consumption order; matmul j-outer over the first two
            # mt-pairs, nt-outer afterwards; evict per PSUM bank ----
            qx = [qxpool.tile([P, 2, TOK_PC], f8, tag="qx", name=f"qx{j}")
                  for j in range(KP)]

            def quant(j, mt0, nmt):
                sl = slice(mt0 * P, (mt0 + nmt) * P)
                nc.vector.tensor_scalar(out=qx[j][:, :, sl],
                                        in0=x_sb[j][:, :, sl],
                                        scalar1=inv_half[:, 0:1],
                                        scalar2=None, op0=Alu.mult)

            def evict(mt, nt, ps):
                ysb = ypool.tile([P, N_TILE], f16, tag="ysb", name="ysb")
                nc.vector.scalar_tensor_tensor(
                    out=ysb[:], in0=ps[:], scalar=out_scale[:, 0:1],
                    in1=bias_bc[:, nt * N_TILE:(nt + 1) * N_TILE],
                    op0=Alu.mult, op1=Alu.add)
                nc.sync.dma_start(
                    y[mt * P:(mt + 1) * P, nt * N_TILE:(nt + 1) * N_TILE],
                    ysb[:])

            def mm(ps, mt, nt, j):
                nc.tensor.matmul(
                    ps[:],
                    qx[j][:, :, mt * P:(mt + 1) * P],
                    qw[j][:, :, nt * N_TILE:(nt + 1) * N_TILE],
                    start=(j == 0), stop=(j == KP - 1),
                    perf_mode=mybir.MatmulPerfMode.DoubleRow)

            # quant for mt0..3, j-major (matches pair j-outer consumption)
            for mts in ((0, 1), (2, 3)):
                for j in range(KP):
                    for mt in mts:
                        quant(j, mt, 1)

            for pi, mts in enumerate(((0, 1), (2, 3))):
                pss = {(mt, nt): psum.tile([P, N_TILE], f32, tag="ps",
                                           name=f"p{mt}_{nt}")
                       for mt in mts for nt in range(NT)}
                for j in range(KP):
                    for mt in mts:
                        for nt in range(NT):
                            mm(pss[(mt, nt)], mt, nt, j)
                for mt in mts:
                    for nt in range(NT):
                        evict(mt, nt, pss[(mt, nt)])
                # look-ahead quant for the next mt-pair(s)
                for mt in (4, 5) if pi == 0 else (6, 7):
                    for j in range(KP):
                        quant(j, mt, 1)

            for mt in range(4, MT):
                for nt in range(NT):
                    ps = psum.tile([P, N_TILE], f32, tag="ps", name=f"ps{nt}")
                    for j in range(KP):
                        mm(ps, mt, nt, j)
                    evict(mt, nt, ps)
                if mt + 4 < MT:
                    for j in range(KP):
                        quant(j, mt + 4, 1)

    nc.compile()
    return nc


def _get_compiled():
    global _compiled
    if _compiled is None:
        _compiled = _build()
    return _compiled


def run(x, weight, bias, **kw):
    """Shard + run on 8 cores; returns (full_output, BassKernelResults)."""
    from concourse.bass_utils import run_bass_kernel_spmd

    nc = _get_compiled()

    x = np.asarray(x, dtype=np.float16)
    weight = np.asarray(weight, dtype=np.float16)
    bias = np.asarray(bias, dtype=np.float16)
    xt = np.ascontiguousarray(x.reshape(TOK, D_IN).T)          # [d_in, tok]
    wt = np.ascontiguousarray(weight.T)                        # [d_in, d_out]
    in_maps = []
    for i in range(N_CORES):
        in_maps.append({
            "xt": np.ascontiguousarray(xt[:, i * TOK_PC:(i + 1) * TOK_PC]),
            "wt": wt,
            "bias": bias,
        })
    res = run_bass_kernel_spmd(nc, in_maps, core_ids=list(range(N_CORES)), **kw)
    out = np.concatenate([res.results[i]["y"] for i in range(N_CORES)], axis=0)
    return out.reshape(B, S, D_OUT), res


def kernel(x, weight, bias):
    out, _ = run(x, weight, bias)
    return out
